# revision 1
# baseline (speedup 1.0000x reference)
"""Two-layer GCN + MLP on 8 Trainium2 NeuronCores.

Math: with A-hat = D^-1/2 (A + I) D^-1/2 and dinv = deg^-1/2,
  h  = relu((A-hat @ x) @ W1 + b1)        (aggregation commutes with W1)
  g  = A-hat @ (h @ W2) + b2
  out = relu(g @ Wm1 + bm1) @ Wm2 + bm2
A-hat @ v = dinv * ((A+I) @ (dinv * v)): row scalings plus an unweighted
gather-sum over edges.  Self-loops are materialized as real (i -> i) edges,
so the aggregation is a pure segment sum; the trailing dinv row scale fuses
into the PSUM->SBUF copy on the scalar engine.  b2 folds into the MLP bias
(bm1_eff = b2 @ Wm1 + bm1) since g only feeds the MLP.

Sharding: nodes (and edges by destination) are split across 8 cores; each
core owns 1250 dst nodes (padded to 1280 = 10 tiles of 128).

The layer-1 gather table y = dinv*x is precomputed on the HOST in fp8e4m3
and fed as an input — no on-device build phase.  Layer 2 needs
t2 = dinv*(h@W2) from all cores: exchanged in fp8 via two AllGathers over
halves of the local t2 buffer (dst tiles 0..T0-1, then T0..9).  Layer-2
edges are split per dst tile into three groups by source location:
L (source on this core: gathered from the local t2 buffer, overlapping the
collectives), A (remote, exchange half 0: overlaps the second AllGather),
and B (remote, half 1: the final phase, followed by the dense MLP).
L and A accumulate into SBUF partials (pa2), pre-scaled by dinv.

Dense layers run transposed in bf16 (psum holds (x@W)^T chunks) so PSUM
copies, relu, and bias fold into scalar-engine activations and no PE
transposes are needed between the two matmuls of each pair.

Edges are sorted by dst tile; per 256-edge pair of chunks the segment sum
is one fp8 DoubleRow PE matmul accumulating into PSUM:
  z[dst_tile] += Ind.T @ G,  Ind[e, d] = (dst_slot[e] == d),  G = gathered rows.
"""
import sys
sys.path.insert(0, "/opt/trn_rl_repo")

import numpy as np
import ml_dtypes

F8NP = ml_dtypes.float8_e4m3

N, D, H = 10000, 256, 512
NC = 8
NSH = N // NC          # 1250 nodes per core
P = 128
NT = 10                # dst tiles per core
NPAD = NT * P          # 1280 padded rows per core
T0 = 5                 # tiles in exchange half 0
NH = (T0 * P, NSH - T0 * P)    # rows per exchange half (pad rows not exchanged)

_cache = {}
_last_res = None
_last_in_maps = None


def _build(key):
    c1, c2 = [list(t) for t in key]   # c1[t]; c2[2*t], c2[2*t+1] = (A, B) chunks
    from concourse import bacc, tile, mybir
    from concourse.masks import make_identity

    f32 = mybir.dt.float32
    bf16 = mybir.dt.bfloat16
    f8 = mybir.dt.float8e4
    i16 = mybir.dt.int16

    off1 = np.concatenate([[0], np.cumsum(c1)]).astype(int)
    off2 = np.concatenate([[0], np.cumsum(c2)]).astype(int)
    NCH1, NCH2 = int(off1[-1]), int(off2[-1])
    L1, L2 = NCH1 * P, NCH2 * P
    gmax1, gmax2 = max(c1), max(c2)

    nc = bacc.Bacc("TRN2", target_bir_lowering=False, debug=False,
                   enable_asserts=True, num_devices=NC)

    ytab = nc.dram_tensor("ytab", [NC * NPAD, D], f8, kind="ExternalInput").ap()
    dinv_d = nc.dram_tensor("dinvown", [P, NT], f32, kind="ExternalInput").ap()
    gidx1 = nc.dram_tensor("gidx1", [P, L1 // 16], i16, kind="ExternalInput").ap()
    dstsel1 = nc.dram_tensor("dstsel1", [P, NCH1], f32, kind="ExternalInput").ap()
    gidx2 = nc.dram_tensor("gidx2", [P, L2 // 16], i16, kind="ExternalInput").ap()
    dstsel2 = nc.dram_tensor("dstsel2", [P, NCH2], f32, kind="ExternalInput").ap()
    iota_d = nc.dram_tensor("iota", [P, P], bf16, kind="ExternalInput").ap()
    W1_d = nc.dram_tensor("W1s", [P, 2, H], bf16, kind="ExternalInput").ap()
    W2_d = nc.dram_tensor("W2s", [P, 4, D], bf16, kind="ExternalInput").ap()
    Wm1_d = nc.dram_tensor("Wm1s", [P, 2, H], bf16, kind="ExternalInput").ap()
    Wm2_d = nc.dram_tensor("Wm2s", [P, 4, D], bf16, kind="ExternalInput").ap()
    b1_d = nc.dram_tensor("b1s", [P, 4], f32, kind="ExternalInput").ap()
    bm1e_d = nc.dram_tensor("bm1es", [P, 4], f32, kind="ExternalInput").ap()
    bm2_d = nc.dram_tensor("bm2row", [1, D], bf16, kind="ExternalInput").ap()
    out_d = nc.dram_tensor("out", [NPAD, D], f32, kind="ExternalOutput").ap()

    out_r = out_d.rearrange("(t p) d -> p t d", p=P)

    with tile.TileContext(nc) as tc:
        with tc.tile_pool(name="const", bufs=1) as cst, \
             tc.tile_pool(name="big", bufs=1) as big, \
             tc.tile_pool(name="gpool", bufs=4) as gpool, \
             tc.tile_pool(name="indp", bufs=4) as indp, \
             tc.tile_pool(name="work", bufs=3) as work, \
             tc.tile_pool(name="pz", bufs=2, space="PSUM") as pz, \
             tc.tile_pool(name="ptr", bufs=2, space="PSUM") as ptr, \
             tc.tile_pool(name="ph", bufs=2, space="PSUM") as ph, \
             tc.tile_pool(name="pt", bufs=2, space="PSUM") as pt, \
             tc.tile_pool(name="dram", bufs=1, space="DRAM") as dram:

            # ------- gather-critical constants first -------
            # gidx1 loads in two halves so tile 0's gather only waits for
            # the first one (subtile deps track the regions separately)
            iota_t = cst.tile([P, P], bf16)
            nc.sync.dma_start(out=iota_t[:], in_=iota_d[:])
            gidx1_t = cst.tile([P, L1 // 16], i16)
            gsplit = (int(off1[5]) * 8 // 2) * 2
            nc.sync.dma_start(out=gidx1_t[:, 0:gsplit], in_=gidx1[:, 0:gsplit])
            dst1_t = cst.tile([P, NCH1], f32)
            nc.sync.dma_start(out=dst1_t[:], in_=dstsel1[:])
            dinv = cst.tile([P, NT], f32)
            nc.sync.dma_start(out=dinv[:], in_=dinv_d[:])
            nc.sync.dma_start(out=gidx1_t[:, gsplit:], in_=gidx1[:, gsplit:])

            def build_ind(dst_t, coff, c, j, out_ap):
                nc.vector.tensor_scalar(
                    out=out_ap, in0=iota_t[:],
                    scalar1=dst_t[:, coff + c + j:coff + c + j + 1],
                    scalar2=None, op0=mybir.AluOpType.is_equal)

            def l1_gather(t):
                nch = c1[t]
                g = gpool.tile([P, nch, D], f8, name="g1", tag="g1",
                               padded_shape=[P, gmax1, D])
                coff = int(off1[t])
                nc.gpsimd.dma_gather(
                    out_ap=g[:], in_ap=ytab[:],
                    idxs_ap=gidx1_t[:, coff * 8:(coff + nch) * 8],
                    num_idxs=nch * P, num_idxs_reg=nch * P, elem_size=D,
                    single_packet=False,
                )
                return g

            g_pending = l1_gather(0)

            # ------- prebuilt one-hot pairs -------
            # Ind tiles depend only on iota/dstsel: build layer-1's ahead of
            # the gathers (so segment matmuls wait only on gather data), and
            # phase B's during the collective window.  One buffer, reused.
            npair1 = NCH1 // 2
            npairB = sum(c2[3 * t + 1] for t in range(NT)) // 2
            indbig = big.tile([P, max(npair1, npairB), 2, P], f8)
            for t in range(NT):
                coff = int(off1[t])
                for c in range(0, c1[t], 2):
                    for j in (0, 1):
                        build_ind(dst1_t, coff, c, j,
                                  indbig[:, (coff + c) // 2, j, :])

            # ------- layer-1 dense constants -------
            W1_t = cst.tile([P, 2, H], bf16)
            W2_t = cst.tile([P, 4, D], bf16)
            b1_t = cst.tile([P, 4], f32)
            with tc.tile_wait_until(0.015):
                nc.sync.dma_start(out=W1_t[:], in_=W1_d[:])
                nc.sync.dma_start(out=W2_t[:], in_=W2_d[:])
                nc.sync.dma_start(out=b1_t[:], in_=b1_d[:])
            ident_t = cst.tile([P, P], bf16)
            make_identity(nc, ident_t[:])
            ones_b = cst.tile([1, P], bf16)
            nc.any.memset(ones_b[:], 1.0)

            pa2 = big.tile([P, NT, D], f32)

            def l2_gather(src_buf, coff, nch):
                g = gpool.tile([P, nch, D], f8, name="g2", tag="g2",
                               padded_shape=[P, gmax2, D])
                nc.gpsimd.dma_gather(
                    out_ap=g[:], in_ap=src_buf[:],
                    idxs_ap=gidx2_t[:, coff * 8:(coff + nch) * 8],
                    num_idxs=nch * P, num_idxs_reg=nch * P, elem_size=D,
                    single_packet=False,
                )
                return g

            def seg_matmuls(psum_z, g, dst_t, coff, nch, start, stop):
                """z += segment-sum of gathered rows for chunk cols [coff, coff+nch).
                nch is even; pairs of chunks run as one fp8 DoubleRow matmul."""
                for c in range(0, nch, 2):
                    ind2 = indp.tile([P, 2, P], f8, name="ind2")
                    for j in (0, 1):
                        build_ind(dst_t, coff, c, j, ind2[:, j, :])
                    nc.tensor.matmul(out=psum_z[:], lhsT=ind2[:, :, :],
                                     rhs=g[:, c:c + 2, :],
                                     perf_mode=mybir.MatmulPerfMode.DoubleRow,
                                     start=(start and c == 0),
                                     stop=(stop and c == nch - 2))

            def seg_matmuls_pre(psum_z, g, indbig, pbase, nch, start, stop):
                """Like seg_matmuls but with prebuilt one-hot pairs in indbig."""
                for c in range(0, nch, 2):
                    nc.tensor.matmul(out=psum_z[:],
                                     lhsT=indbig[:, pbase + c // 2, :, :],
                                     rhs=g[:, c:c + 2, :],
                                     perf_mode=mybir.MatmulPerfMode.DoubleRow,
                                     start=(start and c == 0),
                                     stop=(stop and c == nch - 2))

            def scale_copy(par, out_ap, in_ap, scale_ap):
                if par == 0:
                    nc.scalar.activation(out=out_ap, in_=in_ap,
                                         func=mybir.ActivationFunctionType.Copy,
                                         scale=scale_ap)
                elif par == 1:
                    nc.vector.tensor_scalar(out=out_ap, in0=in_ap,
                                            scalar1=scale_ap, scalar2=None,
                                            op0=mybir.AluOpType.mult)
                else:
                    nc.gpsimd.tensor_scalar(out=out_ap, in0=in_ap,
                                            scalar1=scale_ap, scalar2=None,
                                            op0=mybir.AluOpType.mult)

            def relu_bias(par, out_ap, in_ap, bias_ap):
                if par == 0:
                    nc.scalar.activation(out=out_ap, in_=in_ap,
                                         func=mybir.ActivationFunctionType.Relu,
                                         bias=bias_ap)
                else:
                    nc.vector.tensor_scalar(out=out_ap, in0=in_ap,
                                            scalar1=bias_ap, scalar2=0.0,
                                            op0=mybir.AluOpType.add,
                                            op1=mybir.AluOpType.max)

            def transpose_in(dst, src_ap, nblk, par=0):
                for j in range(nblk):
                    ps = ptr.tile([P, P], bf16, space="PSUM", name="ps_tr")
                    nc.tensor.transpose(out=ps[:], in_=src_ap[:, j * P:(j + 1) * P],
                                        identity=ident_t[:])
                    if par == 0:
                        nc.scalar.activation(
                            out=dst[:, j, :], in_=ps[:],
                            func=mybir.ActivationFunctionType.Copy)
                    else:
                        nc.vector.tensor_copy(out=dst[:, j, :], in_=ps[:])

            def dense_T(psum_hT, W_t, xT, bias_t, act_out, par=0):
                """psum_hT[:, hc, :] = (x @ W)^T chunk; relu+bias into act_out."""
                for hc in range(4):
                    for j in (0, 1):
                        nc.tensor.matmul(
                            out=psum_hT[:, hc, :], lhsT=W_t[:, j, hc * P:(hc + 1) * P],
                            rhs=xT[:, j, :], start=(j == 0), stop=(j == 1))
                    relu_bias(par, act_out[:, hc, :],
                              psum_hT[:, hc, :], bias_t[:, hc:hc + 1])

            t2own = dram.tile([NPAD, D], f8, name="t2own")
            tbuf = [dram.tile([NC * NH[h], D], f8, addr_space="Shared",
                              name=f"tbuf{h}") for h in range(2)]

            def layer1_front(t, g):
                """Segment sum + scaled PSUM copy + transposes -> aggT."""
                psum_z = pz.tile([P, D], f32, space="PSUM", name="psum_z")
                seg_matmuls_pre(psum_z, g, indbig, int(off1[t]) // 2, c1[t],
                                True, True)
                agg1 = work.tile([P, D], bf16, name="agg1")
                scale_copy(0, agg1[:], psum_z[:], dinv[:, t:t + 1])
                aggT = work.tile([P, 2, P], bf16, name="aggT")
                transpose_in(aggT, agg1, 2, 0)
                return aggT

            def layer1_back(t, aggT):
                """Dense W1/W2 + t2 store (emitted one tile behind front so
                its engine-queue entries never stall on same-tile producers)."""
                psum_hT = ph.tile([P, 4, P], f32, space="PSUM", name="psum_h")
                h_sbT = work.tile([P, 4, P], bf16, name="h_sbT")
                dense_T(psum_hT, W1_t, aggT, b1_t, h_sbT, 0)

                psum_t = pt.tile([P, D], f32, space="PSUM", name="psum_t")
                for j in range(4):
                    nc.tensor.matmul(out=psum_t[:], lhsT=h_sbT[:, j, :],
                                     rhs=W2_t[:, j, :],
                                     start=(j == 0), stop=(j == 3))
                # t2 = dinv * (h @ W2), cast to fp8 for the exchange
                t2g = work.tile([P, D], f8, name="t2g")
                scale_copy(0, t2g[:], psum_t[:], dinv[:, t:t + 1])
                nc.sync.dma_start(
                    out=t2own[:].rearrange("(t p) d -> p t d", p=P)[:, t, :],
                    in_=t2g[:])

            aggT_prev = None
            for t in range(NT):
                g = g_pending
                g_pending = l1_gather(t + 1) if t + 1 < NT else None
                aggT_cur = layer1_front(t, g)
                if aggT_prev is not None:
                    layer1_back(t - 1, aggT_prev)
                aggT_prev = aggT_cur
                if t == T0:
                    nc.gpsimd.collective_compute(
                        "AllGather", mybir.AluOpType.bypass,
                        replica_groups=[list(range(NC))],
                        ins=[t2own[0:NH[0], :].opt()], outs=[tbuf[0][:].opt()],
                    )
                if t == 1:
                    # phase-2 tables/weights: loaded in the layer-1 gather gaps
                    gidx2_t = cst.tile([P, L2 // 16], i16)
                    dst2_t = cst.tile([P, NCH2], f32)
                    with tc.tile_wait_until(0.04):
                        nc.sync.dma_start(out=gidx2_t[:], in_=gidx2[:])
                        nc.sync.dma_start(out=dst2_t[:], in_=dstsel2[:])
                elif t == 2:
                    Wm1_t = cst.tile([P, 2, H], bf16)
                    Wm2_t = cst.tile([P, 4, D], bf16)
                    bm1e_t = cst.tile([P, 4], f32)
                    bm2_t = cst.tile([1, D], bf16)
                    with tc.tile_wait_until(0.05):
                        nc.sync.dma_start(out=Wm1_t[:], in_=Wm1_d[:])
                        nc.sync.dma_start(out=Wm2_t[:], in_=Wm2_d[:])
                        nc.sync.dma_start(out=bm1e_t[:], in_=bm1e_d[:])
                        nc.sync.dma_start(out=bm2_t[:], in_=bm2_d[:])
            layer1_back(NT - 1, aggT_prev)
            with tc.high_priority():
                nc.gpsimd.collective_compute(
                    "AllGather", mybir.AluOpType.bypass,
                    replica_groups=[list(range(NC))],
                    ins=[t2own[NH[0]:NH[0] + NH[1], :].opt()], outs=[tbuf[1][:].opt()],
                )

            # ---- phase L: own-core sources, gathered from local t2own
            # (runs in the slack while the AllGathers are in flight)
            with tc.tile_wait_until(0.12):
                gl_pending = l2_gather(t2own, int(off2[2]), c2[2])
                for t in range(NT):
                    g = gl_pending
                    gl_pending = (l2_gather(t2own, int(off2[3 * t + 5]),
                                            c2[3 * t + 5])
                                  if t + 1 < NT else None)
                    psum_z = pz.tile([P, D], f32, space="PSUM", name="psum_z")
                    seg_matmuls(psum_z, g, dst2_t, int(off2[3 * t + 2]),
                                c2[3 * t + 2], True, True)
                    nc.scalar.activation(out=pa2[:, t, :], in_=psum_z[:],
                                         func=mybir.ActivationFunctionType.Copy,
                                         scale=dinv[:, t:t + 1])

            # ---- phase A: half-0 sources for all tiles (overlaps AllGather #2)
            with tc.tile_wait_until(0.12):
                ga_pending = l2_gather(tbuf[0], int(off2[0]), c2[0])
                for t in range(NT):
                    g = ga_pending
                    ga_pending = (l2_gather(tbuf[0], int(off2[3 * t + 3]),
                                            c2[3 * t + 3])
                                  if t + 1 < NT else None)
                    psum_z = pz.tile([P, D], f32, space="PSUM", name="psum_z")
                    seg_matmuls(psum_z, g, dst2_t, int(off2[3 * t]), c2[3 * t],
                                True, True)
                    nc.vector.scalar_tensor_tensor(
                        out=pa2[:, t, :], in0=psum_z[:], scalar=dinv[:, t:t + 1],
                        in1=pa2[:, t, :], op0=mybir.AluOpType.mult,
                        op1=mybir.AluOpType.add)

            # ---- phase B one-hot prebuild (fills the collective window)
            pbB = []
            with tc.tile_wait_until(0.1):
                pb = 0
                for t in range(NT):
                    pbB.append(pb)
                    coff = int(off2[3 * t + 1])
                    for c in range(0, c2[3 * t + 1], 2):
                        for j in (0, 1):
                            build_ind(dst2_t, coff, c, j, indbig[:, pb, j, :])
                        pb += 1

            # ---- phase B: half-1 sources + dense + MLP
            ordB = sorted(range(NT), key=lambda u: -c2[3 * u + 1])
            gb_pending = l2_gather(tbuf[1], int(off2[3 * ordB[0] + 1]),
                                   c2[3 * ordB[0] + 1])
            for i in range(NT):
                t = ordB[i]
                g = gb_pending
                tn = ordB[i + 1] if i + 1 < NT else None
                gb_pending = (l2_gather(tbuf[1], int(off2[3 * tn + 1]),
                                        c2[3 * tn + 1])
                              if tn is not None else None)
                psum_z = pz.tile([P, D], f32, space="PSUM", name="psum_z")
                seg_matmuls_pre(psum_z, g, indbig, pbB[t], c2[3 * t + 1],
                                True, True)
                gagg = work.tile([P, D], bf16, name="gagg")
                nc.vector.scalar_tensor_tensor(
                    out=gagg[:], in0=psum_z[:], scalar=dinv[:, t:t + 1],
                    in1=pa2[:, t, :], op0=mybir.AluOpType.mult,
                    op1=mybir.AluOpType.add)
                par = 0
                gT = work.tile([P, 2, P], bf16, name="gT")
                transpose_in(gT, gagg, 2, par)

                psum_oT = ph.tile([P, 4, P], f32, space="PSUM", name="psum_o",
                                  tag="psum_h")
                o1T = work.tile([P, 4, P], bf16, name="o1T")
                dense_T(psum_oT, Wm1_t, gT, bm1e_t, o1T, par)

                psum_y = pt.tile([P, D], f32, space="PSUM", name="psum_y",
                                 tag="psum_t")
                for j in range(4):
                    nc.tensor.matmul(out=psum_y[:], lhsT=o1T[:, j, :],
                                     rhs=Wm2_t[:, j, :],
                                     start=(j == 0), stop=False)
                nc.tensor.matmul(out=psum_y[:], lhsT=ones_b[:1, :], rhs=bm2_t[:1, :],
                                 start=False, stop=True)
                out_sb = work.tile([P, D], f32, name="out_sb")
                if par == 0:
                    nc.scalar.activation(out=out_sb[:], in_=psum_y[:],
                                         func=mybir.ActivationFunctionType.Copy)
                else:
                    nc.vector.tensor_copy(out=out_sb[:], in_=psum_y[:])
                nc.sync.dma_start(out=out_r[:, t, :], in_=out_sb[:])

    nc.finalize()
    return nc


def _wrap16(flat):
    """edge list -> dma_gather int16 index layout [128, len/16]."""
    arr16 = flat.reshape(-1, 16)
    return np.tile(arr16.T, (8, 1)).astype(np.int16)


def _even_ceil(n):
    c = -(-int(n) // P)
    return c + (c & 1)


def _prep(edge_index):
    """Host graph preprocessing: degrees + per-core padded edge lists.

    Self-loops are appended as real edges.  Layer 1 indexes the full padded
    y table [8*1280]; layer 2 edges are split per dst tile into (A, B)
    groups by which exchange half holds the source, indexing tbuf half
    layouts [8*NH[h]].  Chunk counts are maxed across cores (the SPMD
    program is shared) and rounded up to EVEN (DoubleRow chunk pairs).
    Returns (deg, per-core arrays, (c1, c2) build key).
    """
    src_e = np.asarray(edge_index[0], dtype=np.int64)
    dst_e = np.asarray(edge_index[1], dtype=np.int64)
    deg = 1 + np.bincount(dst_e, minlength=N).astype(np.int64)
    loops = np.arange(N, dtype=np.int64)
    src = np.concatenate([src_e, loops])
    dst = np.concatenate([dst_e, loops])

    shard = dst // NSH
    tile_g = (dst - shard * NSH) // P      # dst tile within core (0..9)
    slot = (dst - shard * NSH) % P
    sshard = src // NSH
    soff = src - sshard * NSH
    src1 = sshard * NPAD + soff            # full-y row
    half = (soff >= NH[0]).astype(np.int64)  # exchange half of the source
    src2r = sshard * NH[0] + soff            # tbuf[half] row for remote srcs
    src2r[half == 1] = sshard[half == 1] * NH[1] + (soff[half == 1] - NH[0])

    # per-core groups: 0 = A (remote, half 0), 1 = B (remote, half 1),
    #                  2 = L (source on this core -> local t2own row)
    counts1 = np.zeros((NC, NT), np.int64)
    counts2 = np.zeros((NC, NT, 3), np.int64)
    per_core = []
    for k in range(NC):
        sel = shard == k
        t_k = tile_g[sel]
        local = (sshard[sel] == k)
        g_k = np.where(local, 2, half[sel])
        s2_k = np.where(local, soff[sel], src2r[sel])
        order = np.lexsort((g_k, t_k))
        e = dict(src1=src1[sel][order], src2=s2_k[order],
                 slot=slot[sel][order], t=t_k[order], g=g_k[order])
        per_core.append(e)
        counts1[k] = np.bincount(t_k, minlength=NT)
        for t in range(NT):
            counts2[k, t] = np.bincount(g_k[t_k == t], minlength=3)

    c1 = tuple(_even_ceil(counts1[:, t].max()) for t in range(NT))
    c2 = []
    for t in range(NT):
        for g in range(3):
            cg = _even_ceil(counts2[:, t, g].max())
            assert cg > 0, (t, g)
            c2.append(cg)
    c2 = tuple(c2)
    key = (c1, c2)

    off1 = np.concatenate([[0], np.cumsum(c1)]).astype(int)
    off2 = np.concatenate([[0], np.cumsum(c2)]).astype(int)
    L1, L2 = int(off1[-1]) * P, int(off2[-1]) * P

    arrays = []
    for k in range(NC):
        e = per_core[k]
        idx1 = np.zeros(L1, np.int16)
        sel1 = np.full(L1, -1.0, np.float32)
        idx2 = np.zeros(L2, np.int16)
        sel2 = np.full(L2, -1.0, np.float32)
        pos = 0
        for t in range(NT):
            n = int(counts1[k, t])
            seg = slice(pos, pos + n)
            base1 = int(off1[t]) * P
            idx1[base1:base1 + n] = e["src1"][seg]
            sel1[base1:base1 + n] = e["slot"][seg]
            gpos = 0
            for g in range(3):
                ng = int(counts2[k, t, g])
                baseg = int(off2[3 * t + g]) * P
                idx2[baseg:baseg + ng] = e["src2"][seg][gpos:gpos + ng]
                sel2[baseg:baseg + ng] = e["slot"][seg][gpos:gpos + ng]
                gpos += ng
            pos += n
        arrays.append(dict(
            gidx1=_wrap16(idx1),
            dstsel1=np.ascontiguousarray(sel1.reshape(-1, P).T),
            gidx2=_wrap16(idx2),
            dstsel2=np.ascontiguousarray(sel2.reshape(-1, P).T),
        ))
    return deg, arrays, key


def _make_in_maps(x, edge_index, W1, b1, W2, b2, Wm1, bm1, Wm2, bm2):
    x = np.asarray(x, dtype=np.float32)
    deg, arrays, key = _prep(edge_index)
    bf16 = ml_dtypes.bfloat16
    iota = np.tile(np.arange(P, dtype=np.float32), (P, 1)).astype(bf16)
    W1s = np.ascontiguousarray(
        np.asarray(W1, np.float32).reshape(2, P, H).transpose(1, 0, 2)).astype(bf16)
    W2s = np.ascontiguousarray(
        np.asarray(W2, np.float32).reshape(4, P, D).transpose(1, 0, 2)).astype(bf16)
    Wm1s = np.ascontiguousarray(
        np.asarray(Wm1, np.float32).reshape(2, P, H).transpose(1, 0, 2)).astype(bf16)
    Wm2s = np.ascontiguousarray(
        np.asarray(Wm2, np.float32).reshape(4, P, D).transpose(1, 0, 2)).astype(bf16)
    b1s = np.ascontiguousarray(np.asarray(b1, np.float32).reshape(4, P).T)
    bm1e = (np.asarray(b2, np.float32) @ np.asarray(Wm1, np.float32)
            + np.asarray(bm1, np.float32))
    bm1es = np.ascontiguousarray(bm1e.reshape(4, P).T.astype(np.float32))
    bm2row = np.asarray(bm2, np.float32).reshape(1, D).astype(bf16)

    dinv = (1.0 / np.sqrt(deg.astype(np.float32))).astype(np.float32)
    yf = np.zeros((NC, NPAD, D), np.float32)
    dinvf = np.zeros((NC, NPAD), np.float32)
    for k in range(NC):
        yf[k, :NSH] = x[k * NSH:(k + 1) * NSH] * dinv[k * NSH:(k + 1) * NSH, None]
        dinvf[k, :NSH] = dinv[k * NSH:(k + 1) * NSH]
    ytab = yf.reshape(NC * NPAD, D).astype(F8NP)

    in_maps = []
    for k in range(NC):
        dinvown = np.ascontiguousarray(dinvf[k].reshape(NT, P).T)
        in_maps.append(dict(
            ytab=ytab, dinvown=dinvown,
            iota=iota, W1s=W1s, W2s=W2s, Wm1s=Wm1s, Wm2s=Wm2s,
            b1s=b1s, bm1es=bm1es, bm2row=bm2row,
            **arrays[k],
        ))

    return in_maps, key


def kernel(x, edge_index, W1, b1, W2, b2, Wm1, bm1, Wm2, bm2):
    from concourse.bass_utils import run_bass_kernel_spmd

    in_maps, key = _make_in_maps(x, edge_index, W1, b1, W2, b2,
                                 Wm1, bm1, Wm2, bm2)
    if key not in _cache:
        _cache[key] = _build(key)
    nc = _cache[key]

    global _last_res, _last_in_maps
    _last_in_maps = in_maps
    res = run_bass_kernel_spmd(nc, in_maps, core_ids=list(range(NC)))
    _last_res = res
    out = np.concatenate(
        [res.results[k]["out"][:NSH] for k in range(NC)], axis=0)
    return out.astype(np.float32)



# revision 2
# speedup vs baseline: 2.2028x; 2.2028x over previous
"""Two-layer GCN + MLP on 8 Trainium2 NeuronCores — dense-A formulation, v2.

Same math/sharding as kernel_dense (see its docstring): exact fp8 (A+I)
multiplicity matrix, SBUF-resident, DoubleRow matmuls for both GCN
aggregations; y = dinv*x precomputed fp8 on host; two AllGathers exchange
t2 = dinv*(h@W2).

v2 cuts DMA count ~64 -> ~20 (per-DMA fixed cost is ~1us engine-serial):
weights packed into 2 inputs, t2own/out stores staged in SBUF and issued
as half-range DMAs, tbuf loads merged to one 4D-AP DMA per half.  Loads
ride the SP queue; stores + tbuf loads ride the DVE queue so they don't
queue behind the A stream.  Transpose PSUM->SBUF copies also move to DVE
to unload the scalar engine.
"""
import sys
sys.path.insert(0, "/opt/trn_rl_repo")

import os
import numpy as np
import ml_dtypes

NOAG = bool(os.environ.get("ABL_NOAG"))  # timing probe: skip collectives

F8NP = ml_dtypes.float8_e4m3
BF16 = ml_dtypes.bfloat16

N, D, H = 10000, 256, 512
NC = 8
NSH = N // NC          # 1250 nodes per core
P = 128
NT = 10                # dst tiles per core
NPAD = NT * P          # 1280 padded rows per core
SC = NC * NT           # 80 global src chunks
NPR = SC // 2          # 40 chunk pairs
T0 = 4                 # dst tiles in exchange half 0
H0R = T0 * P           # 512 rows in half 0
H1R = NPAD - H0R       # 768 rows in half 1
SLC = 16               # src chunks per A-load slice
# global src-chunk permutation: exchange-half-0 chunks (each core's dst
# tiles 0..3) first, then half-1 chunks -- makes both tbuf->SBUF loads a
# single contiguous 3D-AP DMA and both layer-2 phases contiguous pair
# ranges.  Applied host-side to A and y chunk dims (core-independent).
PERM = ([NT * j + c for j in range(NC) for c in range(T0)]
        + [NT * j + T0 + c for j in range(NC) for c in range(NT - T0)])
NCH0 = NC * T0         # 32 half-0 chunks
PAIRS_H0 = list(range(NCH0 // 2))
PAIRS_H1 = list(range(NCH0 // 2, NPR))
# packed bf16 weight offsets (cols)
OW1, OW2, OM1, OM2, OB2 = 0, 1024, 2048, 3072, 4096
WBF_COLS = 4352
# packed f32 offsets
OB1, OBM, ODI = 0, 4, 8
WF_COLS = 18

_cache = {}
_last_res = None
_last_in_maps = None


def _build():
    from concourse import bacc, tile, mybir
    from concourse.masks import make_identity

    f32 = mybir.dt.float32
    bf16 = mybir.dt.bfloat16
    f8 = mybir.dt.float8e4
    DR = mybir.MatmulPerfMode.DoubleRow
    ACT = mybir.ActivationFunctionType

    nc = bacc.Bacc("TRN2", target_bir_lowering=False, debug=False,
                   enable_asserts=True, num_devices=NC)

    A_d = nc.dram_tensor("Ashard", [P, SC, NPAD], f8, kind="ExternalInput").ap()
    y_d = nc.dram_tensor("ytabs", [P, SC, D], f8, kind="ExternalInput").ap()
    wbf_d = nc.dram_tensor("wbf", [P, WBF_COLS], bf16, kind="ExternalInput").ap()
    wf_d = nc.dram_tensor("wf", [P, WF_COLS], f32, kind="ExternalInput").ap()
    out_d = nc.dram_tensor("out", [NPAD, D], f32, kind="ExternalOutput").ap()

    out_r = out_d.rearrange("(t p) d -> p t d", p=P)

    with tile.TileContext(nc) as tc:
        with tc.tile_pool(name="cst", bufs=1) as cst, \
             tc.tile_pool(name="big", bufs=1) as big, \
             tc.tile_pool(name="work", bufs=3) as work, \
             tc.tile_pool(name="pz", bufs=5, space="PSUM") as pz, \
             tc.tile_pool(name="ptr", bufs=1, space="PSUM") as ptr, \
             tc.tile_pool(name="ph", bufs=1, space="PSUM") as ph, \
             tc.tile_pool(name="pt", bufs=1, space="PSUM") as pt, \
             tc.tile_pool(name="dram", bufs=1, space="DRAM") as dram:

            A_sb = big.tile([P, SC, NPAD], f8, name="A_sb")
            y_sb = big.tile([P, SC, D], f8, name="y_sb")
            t2_sb = big.tile([P, SC, D], f8, name="t2_sb")
            t2st = big.tile([P, NT, D], f8, name="t2st")
            ost = big.tile([P, NT, D], f32, name="ost")
            pa2 = big.tile([P, NT, D], f32, name="pa2")

            wf_t = cst.tile([P, WF_COLS], f32)
            wbf_t = cst.tile([P, WBF_COLS], bf16)
            # SP queue: y head, consts, A slices streaming
            nc.sync.dma_start(out=y_sb[:, 0:SLC, :], in_=y_d[:, 0:SLC, :])
            nc.sync.dma_start(out=wf_t[:], in_=wf_d[:])
            nc.sync.dma_start(out=A_sb[:, 0:SLC, :], in_=A_d[:, 0:SLC, :])
            nc.sync.dma_start(out=wbf_t[:], in_=wbf_d[:])
            nc.sync.dma_start(out=y_sb[:, SLC:SC, :], in_=y_d[:, SLC:SC, :])
            for s in range(1, SC // SLC):
                nc.sync.dma_start(out=A_sb[:, s * SLC:(s + 1) * SLC, :],
                                  in_=A_d[:, s * SLC:(s + 1) * SLC, :])

            ident_t = cst.tile([P, P], bf16)
            make_identity(nc, ident_t[:])
            ones_b = cst.tile([1, P], bf16)
            nc.any.memset(ones_b[:], 1.0)

            dinv = wf_t[:, ODI:ODI + NT]

            t2own = dram.tile([NPAD, D], f8, name="t2own")
            t2own_r = t2own[:].rearrange("(t p) d -> p t d", p=P)
            tbuf0 = dram.tile([NC * H0R, D], f8, addr_space="Shared",
                              name="tbuf0")
            tbuf1 = dram.tile([NC * H1R, D], f8, addr_space="Shared",
                              name="tbuf1")

            def agg_mm(z, pr, t, rhs_sb, start, stop):
                nc.tensor.matmul(
                    out=z[:], lhsT=A_sb[:, 2 * pr:2 * pr + 2, t * P:(t + 1) * P],
                    rhs=rhs_sb[:, 2 * pr:2 * pr + 2, :],
                    perf_mode=DR, start=start, stop=stop)

            def transpose_in(dst, src_ap, nblk):
                for j in range(nblk):
                    ps = ptr.tile([P, P], bf16, name="ps_tr")
                    nc.tensor.transpose(out=ps[:], in_=src_ap[:, j * P:(j + 1) * P],
                                        identity=ident_t[:])
                    nc.vector.tensor_copy(out=dst[:, j, :], in_=ps[:])

            def dense_T(psum_hT, w_off, xT, b_off, act_out):
                for hc in range(4):
                    for j in (0, 1):
                        nc.tensor.matmul(
                            out=psum_hT[:, hc, :],
                            lhsT=wbf_t[:, w_off + j * 512 + hc * P:
                                       w_off + j * 512 + (hc + 1) * P],
                            rhs=xT[:, j, :], start=(j == 0), stop=(j == 1))
                    nc.scalar.activation(out=act_out[:, hc, :],
                                         in_=psum_hT[:, hc, :],
                                         func=ACT.Relu,
                                         bias=wf_t[:, b_off + hc:b_off + hc + 1])

            def dense_tile(t, z):
                """dinv-scale agg, W1+relu, W2, dinv-scale -> t2st."""
                agg1 = work.tile([P, D], bf16, name="agg1")
                nc.scalar.activation(out=agg1[:], in_=z[:], func=ACT.Copy,
                                     scale=dinv[:, t:t + 1])
                aggT = work.tile([P, 2, P], bf16, name="aggT")
                transpose_in(aggT, agg1, 2)
                psum_hT = ph.tile([P, 4, P], f32, name="psum_h")
                h_sbT = work.tile([P, 4, P], bf16, name="h_sbT")
                dense_T(psum_hT, OW1, aggT, OB1, h_sbT)
                psum_t = pt.tile([P, D], f32, name="psum_t")
                for j in range(4):
                    nc.tensor.matmul(
                        out=psum_t[:], lhsT=h_sbT[:, j, :],
                        rhs=wbf_t[:, OW2 + j * D:OW2 + (j + 1) * D],
                        start=(j == 0), stop=(j == 3))
                nc.scalar.activation(out=t2st[:, t, :], in_=psum_t[:],
                                     func=ACT.Copy, scale=dinv[:, t:t + 1])

            # ---- layer 1, wave A (tiles 0..4): slice-major streaming
            WA = 5
            zs = [pz.tile([P, D], f32, name="z") for _ in range(WA)]
            for s in range(SC // SLC):
                for t in range(WA):
                    for pr in range(s * SLC // 2, (s + 1) * SLC // 2):
                        agg_mm(zs[t], pr, t, y_sb,
                               start=(pr == 0), stop=(pr == NPR - 1))
            for t in range(WA):
                dense_tile(t, zs[t])
                if t == T0 - 1:
                    nc.scalar.dma_start(out=t2own_r[:, 0:T0, :],
                                        in_=t2st[:, 0:T0, :])
                    if not NOAG:
                        nc.gpsimd.collective_compute(
                            "AllGather", mybir.AluOpType.bypass,
                            replica_groups=[list(range(NC))],
                            ins=[t2own[0:H0R, :].opt()], outs=[tbuf0[:].opt()],
                        )

            # ---- layer 1, wave B (tiles 5..9): A fully resident
            for t in range(WA, NT):
                z = pz.tile([P, D], f32, name="z")
                for pr in range(NPR):
                    agg_mm(z, pr, t, y_sb, start=(pr == 0), stop=(pr == NPR - 1))
                dense_tile(t, z)
            nc.scalar.dma_start(out=t2own_r[:, T0:NT, :], in_=t2st[:, T0:NT, :])
            if not NOAG:
                with tc.high_priority():
                    nc.gpsimd.collective_compute(
                        "AllGather", mybir.AluOpType.bypass,
                        replica_groups=[list(range(NC))],
                        ins=[t2own[H0R:NPAD, :].opt()], outs=[tbuf1[:].opt()],
                    )

            # ---- exchange half 0 -> t2_sb chunks [0:32], one DMA
            nc.scalar.dma_start(
                out=t2_sb[:, 0:NCH0, :],
                in_=tbuf0[:].rearrange("(c p) d -> p c d", p=P))

            # ---- layer 2 phase H0
            for t in range(NT):
                z = pz.tile([P, D], f32, name="z")
                for i, pr in enumerate(PAIRS_H0):
                    agg_mm(z, pr, t, t2_sb, start=(i == 0),
                           stop=(i == len(PAIRS_H0) - 1))
                nc.scalar.activation(out=pa2[:, t, :], in_=z[:], func=ACT.Copy,
                                     scale=dinv[:, t:t + 1])

            # ---- exchange half 1 -> t2_sb chunks [32:80], one DMA
            nc.scalar.dma_start(
                out=t2_sb[:, NCH0:SC, :],
                in_=tbuf1[:].rearrange("(c p) d -> p c d", p=P))

            # ---- layer 2 phase H1 + MLP per tile
            for t in range(NT):
                z = pz.tile([P, D], f32, name="z")
                for i, pr in enumerate(PAIRS_H1):
                    agg_mm(z, pr, t, t2_sb, start=(i == 0),
                           stop=(i == len(PAIRS_H1) - 1))
                gagg = work.tile([P, D], bf16, name="gagg")
                nc.vector.scalar_tensor_tensor(
                    out=gagg[:], in0=z[:], scalar=dinv[:, t:t + 1],
                    in1=pa2[:, t, :], op0=mybir.AluOpType.mult,
                    op1=mybir.AluOpType.add)
                gT = work.tile([P, 2, P], bf16, name="gT")
                transpose_in(gT, gagg, 2)
                psum_oT = ph.tile([P, 4, P], f32, name="psum_h")
                o1T = work.tile([P, 4, P], bf16, name="o1T")
                dense_T(psum_oT, OM1, gT, OBM, o1T)
                psum_y = pt.tile([P, D], f32, name="psum_t")
                for j in range(4):
                    nc.tensor.matmul(
                        out=psum_y[:], lhsT=o1T[:, j, :],
                        rhs=wbf_t[:, OM2 + j * D:OM2 + (j + 1) * D],
                        start=(j == 0), stop=False)
                nc.tensor.matmul(out=psum_y[:], lhsT=ones_b[:1, :],
                                 rhs=wbf_t[0:1, OB2:OB2 + D],
                                 start=False, stop=True)
                nc.scalar.activation(out=ost[:, t, :], in_=psum_y[:],
                                     func=ACT.Copy)
                if t == 4:
                    nc.scalar.dma_start(out=out_r[:, 0:5, :], in_=ost[:, 0:5, :])
            nc.scalar.dma_start(out=out_r[:, 5:NT, :], in_=ost[:, 5:NT, :])

    nc.finalize()
    return nc


def _make_in_maps(x, edge_index, W1, b1, W2, b2, Wm1, bm1, Wm2, bm2):
    x = np.asarray(x, dtype=np.float32)
    src = np.asarray(edge_index[0], dtype=np.int64)
    dst = np.asarray(edge_index[1], dtype=np.int64)
    deg = 1 + np.bincount(dst, minlength=N).astype(np.int64)
    dinv = (1.0 / np.sqrt(deg.astype(np.float32))).astype(np.float32)

    # dense multiplicity matrix (A + I) in padded global coordinates;
    # small-int entries are exact in fp8e4m3 (bit patterns via LUT)
    srcp = (src // NSH) * NPAD + (src % NSH)
    dstp = (dst // NSH) * NPAD + (dst % NSH)
    GP = NC * NPAD
    mcnt = np.zeros((GP, GP), np.uint8)
    np.add.at(mcnt, (srcp, dstp), 1)
    rp = (np.arange(N) // NSH) * NPAD + (np.arange(N) % NSH)
    mcnt[rp, rp] += 1
    lut = np.arange(256, dtype=np.float32).astype(F8NP).view(np.uint8)
    abits = lut[mcnt]

    yf = np.zeros((NC, NPAD, D), np.float32)
    dinvf = np.zeros((NC, NPAD), np.float32)
    for k in range(NC):
        yf[k, :NSH] = x[k * NSH:(k + 1) * NSH] * dinv[k * NSH:(k + 1) * NSH, None]
        dinvf[k, :NSH] = dinv[k * NSH:(k + 1) * NSH]
    y8 = yf.reshape(GP, D).astype(F8NP)
    ytabs = np.ascontiguousarray(
        y8.reshape(SC, P, D)[PERM].transpose(1, 0, 2))

    bf16 = BF16
    wbf = np.zeros((P, WBF_COLS), bf16)
    wbf[:, OW1:OW1 + 1024] = np.asarray(W1, np.float32).reshape(
        2, P, H).transpose(1, 0, 2).reshape(P, 1024).astype(bf16)
    wbf[:, OW2:OW2 + 1024] = np.asarray(W2, np.float32).reshape(
        4, P, D).transpose(1, 0, 2).reshape(P, 1024).astype(bf16)
    wbf[:, OM1:OM1 + 1024] = np.asarray(Wm1, np.float32).reshape(
        2, P, H).transpose(1, 0, 2).reshape(P, 1024).astype(bf16)
    wbf[:, OM2:OM2 + 1024] = np.asarray(Wm2, np.float32).reshape(
        4, P, D).transpose(1, 0, 2).reshape(P, 1024).astype(bf16)
    wbf[:, OB2:OB2 + D] = np.asarray(bm2, np.float32).reshape(1, D).astype(bf16)

    bm1e = (np.asarray(b2, np.float32) @ np.asarray(Wm1, np.float32)
            + np.asarray(bm1, np.float32))

    in_maps = []
    for k in range(NC):
        ashard = np.ascontiguousarray(
            abits[:, k * NPAD:(k + 1) * NPAD].reshape(SC, P, NPAD)[PERM]
            .transpose(1, 0, 2)).view(F8NP)
        wf = np.zeros((P, WF_COLS), np.float32)
        wf[:, OB1:OB1 + 4] = np.asarray(b1, np.float32).reshape(4, P).T
        wf[:, OBM:OBM + 4] = bm1e.reshape(4, P).T
        wf[:, ODI:ODI + NT] = dinvf[k].reshape(NT, P).T
        in_maps.append(dict(Ashard=ashard, ytabs=ytabs, wbf=wbf, wf=wf))
    return in_maps, ()


def kernel(x, edge_index, W1, b1, W2, b2, Wm1, bm1, Wm2, bm2):
    from concourse.bass_utils import run_bass_kernel_spmd

    in_maps, key = _make_in_maps(x, edge_index, W1, b1, W2, b2,
                                 Wm1, bm1, Wm2, bm2)
    if key not in _cache:
        _cache[key] = _build()
    nc = _cache[key]

    global _last_res, _last_in_maps
    _last_in_maps = in_maps
    res = run_bass_kernel_spmd(nc, in_maps, core_ids=list(range(NC)))
    _last_res = res
    out = np.concatenate(
        [res.results[k]["out"][:NSH] for k in range(NC)], axis=0)
    return out.astype(np.float32)
